# revision 46
# baseline (speedup 1.0000x reference)
"""BiLevelRoutingAttention (spiking, linear attention with window routing) on 8 TRN2 cores.

Sharding: 16 (t,b) pairs -> 2 per core, data-parallel. Host precomputes routing
(region sums -> top-k window indices -> gather row indices); the device does
qkv projection (3-term f32r residual-split for fp32-grade accuracy, thresholds
fused into paired-PSUM-bank evacuations), per-window kv outer products (fp8
DoubleRow on the binary spikes), top-k aggregation via indirect-DMA row gathers
with CCE-add accumulation (runs on the otherwise-idle gpsimd DMA queue),
linear attention (bf16), and the output projection (2-term: f32r main + fp8
DoubleRow residual at a 2^9 PSUM scale), producing binary spikes in fp8 that
the host converts back to f32 and transposes. The two pairs per core are
software-pipelined so the tensor engine never drains.
"""
import sys
sys.path.insert(0, '/opt/trn_rl_repo')

import numpy as np
import ml_dtypes

import concourse.bass as bass
import concourse.bacc as bacc
import concourse.mybir as mybir
from concourse.tile import TileContext
from concourse import bass_utils

F32 = mybir.dt.float32
F32R = mybir.dt.float32r
BF16 = mybir.dt.bfloat16
F8 = mybir.dt.float8e4
F16 = mybir.dt.float16
I32 = mybir.dt.int32
GE = mybir.AluOpType.is_ge
MULT = mybir.AluOpType.mult
ADD = mybir.AluOpType.add
SIG = mybir.ActivationFunctionType.Sigmoid
COPYF = mybir.ActivationFunctionType.Copy
DRMODE = mybir.MatmulPerfMode.DoubleRow

T, B, L, C = 4, 4, 4096, 256
NW, TOPK, H, D = 8, 4, 4, 64
WIN = L // NW           # 512
NCORES = 8
NPAIR = 2               # (t,b) pairs per core
BIGS = 1.0e18           # sigmoid saturation scale
PS = 512.0              # proj main-term PSUM scale (= 2^-5 * 2^14)

_EXEC_TIME_NS = None    # stashed for test harness


def _ensure_ntff_hook():
    """The agent image's antenv lacks axon_hooks; register the same hook
    trn_boot would have installed so trace=True can collect NTFF profiles."""
    import types
    try:
        import antenv.axon_hooks  # noqa: F401
        return True
    except ImportError:
        pass
    try:
        import antenv
        from trn_agent_boot.trn_boot import _ntff_profile_via_ctypes
        state = {"hook": _ntff_profile_via_ctypes('/opt/axon/libaxon_pjrt.so')}
        mod = types.ModuleType("antenv.axon_hooks")
        mod.get_axon_ntff_profile_hook = lambda: state["hook"]
        mod.set_axon_ntff_profile_hook = lambda h: state.__setitem__("hook", h)
        sys.modules["antenv.axon_hooks"] = mod
        antenv.axon_hooks = mod
        return True
    except Exception:
        return False


def _build_nc():
    nc = bacc.Bacc("TRN2", target_bir_lowering=False, debug=False,
                   num_devices=8)

    xt = nc.dram_tensor("xt", [NPAIR, C, L], F32, kind="ExternalInput")
    xtb = nc.dram_tensor("xtb", [NPAIR, C, L], F16, kind="ExternalInput")
    wqkv = nc.dram_tensor("wqkv", [C, 768], F32, kind="ExternalInput")
    wqv = nc.dram_tensor("wqv", [C, 768], F32, kind="ExternalInput")
    wkv16 = nc.dram_tensor("wkv16", [C, 512], F16, kind="ExternalInput")
    wq16 = nc.dram_tensor("wq16", [C, 256], F16, kind="ExternalInput")
    thrkv2 = nc.dram_tensor("thrkv2", [128, 1024], F32, kind="ExternalInput")
    thrq = nc.dram_tensor("thrq", [C, 1], F32, kind="ExternalInput")
    sigbq = nc.dram_tensor("sigbq", [C, 1], F32, kind="ExternalInput")
    wps = nc.dram_tensor("wps", [C, C], F32, kind="ExternalInput")
    wpl8 = nc.dram_tensor("wpl8", [128, 512], F8, kind="ExternalInput")
    thrp = nc.dram_tensor("thrp", [C, 1], F32, kind="ExternalInput")
    sigbp = nc.dram_tensor("sigbp", [C, 1], F32, kind="ExternalInput")
    idxrow = nc.dram_tensor("idxrow", [NPAIR, 128, NW * TOPK], I32,
                            kind="ExternalInput")
    maskr = nc.dram_tensor("maskr", [128, NW * NW], F32, kind="ExternalInput")
    out = nc.dram_tensor("out", [NPAIR, C, L], F8, kind="ExternalOutput")
    kvw_dram = [nc.dram_tensor("kvw_scratch0", [NW * 128, 256], BF16,
                               kind="Internal"),
                nc.dram_tensor("kvw_scratch1", [NW * 128, 256], BF16,
                               kind="Internal")]

    with TileContext(nc) as tc:
        with (
            tc.tile_pool(name="const", bufs=1) as cpool,
            tc.tile_pool(name="xtp", bufs=2) as xtp,
            tc.tile_pool(name="big", bufs=1) as big,
            tc.tile_pool(name="dbl", bufs=2) as dbl,
            tc.tile_pool(name="fin", bufs=4) as finp,
            tc.tile_pool(name="psW", bufs=2, space="PSUM") as psW,
            tc.tile_pool(name="psN", bufs=2, space="PSUM") as psN,
            tc.tile_pool(name="psB", bufs=2, space="PSUM") as psB,
        ):
            # ---- constants / weights ----
            # k/v columns (needed by phase A immediately) in their own tiles;
            # q columns (needed only by qT, much later) loaded separately.
            wkv_sb = [cpool.tile([128, 512], F32R, tag="wkv0", name="wkv0"),
                      cpool.tile([128, 512], F32R, tag="wkv1", name="wkv1")]
            nc.gpsimd.dma_start(wkv_sb[0][:], wqkv[0:128, 256:768])
            nc.gpsimd.dma_start(wkv_sb[1][:], wqkv[128:256, 256:768])
            wvkv_sb = [cpool.tile([128, 512], F32R, tag="wvkv0", name="wvkv0"),
                       cpool.tile([128, 512], F32R, tag="wvkv1", name="wvkv1")]
            nc.gpsimd.dma_start(wvkv_sb[0][:], wqv[0:128, 256:768])
            nc.gpsimd.dma_start(wvkv_sb[1][:], wqv[128:256, 256:768])
            wkv16_sb = [cpool.tile([128, 512], F16, tag="wkv16_0", name="wkv16_0"),
                        cpool.tile([128, 512], F16, tag="wkv16_1", name="wkv16_1")]
            nc.gpsimd.dma_start(wkv16_sb[0][:], wkv16[0:128, :])
            nc.gpsimd.dma_start(wkv16_sb[1][:], wkv16[128:256, :])
            thrkv_sb = cpool.tile([128, 1024], F32, tag="thrkv", name="thrkv")
            nc.gpsimd.dma_start(thrkv_sb[:], thrkv2[:])
            w_sb = [cpool.tile([128, 256], F32R, tag="wq0", name="wq0"),
                    cpool.tile([128, 256], F32R, tag="wq1", name="wq1")]
            wv_sb = [cpool.tile([128, 256], F32R, tag="wv0", name="wv0"),
                     cpool.tile([128, 256], F32R, tag="wv1", name="wv1")]
            wq16_sb = [cpool.tile([128, 256], F16, tag="wq16_0", name="wq16_0"),
                       cpool.tile([128, 256], F16, tag="wq16_1", name="wq16_1")]
            wp_sb = [cpool.tile([128, 256], F32R, tag="wp0", name="wp0"),
                     cpool.tile([128, 256], F32R, tag="wp1", name="wp1")]
            wpl8_sb = cpool.tile([128, 512], F8, tag="wpl8", name="wpl8")
            thrq_sb = cpool.tile([128, 2], F32, tag="thrq", name="thrq")
            sigbq_sb = cpool.tile([128, 2], F32, tag="sigbq", name="sigbq")
            thrp_sb = cpool.tile([128, 2], F32, tag="thrp", name="thrp")
            sigbp_sb = cpool.tile([128, 2], F32, tag="sigbp", name="sigbp")
            idx_sb = [cpool.tile([128, NW * TOPK], I32, tag="idx0", name="idx0"),
                      cpool.tile([128, NW * TOPK], I32, tag="idx1", name="idx1")]
            mask_sb = cpool.tile([128, NW * NW], F32, tag="maskr", name="maskr")

            def load_consts_rest():
                nc.gpsimd.dma_start(w_sb[0][:], wqkv[0:128, 0:256])
                nc.gpsimd.dma_start(w_sb[1][:], wqkv[128:256, 0:256])
                nc.gpsimd.dma_start(wv_sb[0][:], wqv[0:128, 0:256])
                nc.gpsimd.dma_start(wv_sb[1][:], wqv[128:256, 0:256])
                nc.gpsimd.dma_start(wq16_sb[0][:], wq16[0:128, :])
                nc.gpsimd.dma_start(wq16_sb[1][:], wq16[128:256, :])
                nc.gpsimd.dma_start(wp_sb[0][:], wps[0:128, :])
                nc.gpsimd.dma_start(wp_sb[1][:], wps[128:256, :])
                nc.gpsimd.dma_start(wpl8_sb[:], wpl8[:])
                nc.sync.dma_start(thrq_sb[:],
                                  thrq.rearrange("(a p) b -> p (a b)", p=128))
                nc.sync.dma_start(sigbq_sb[:],
                                  sigbq.rearrange("(a p) b -> p (a b)", p=128))
                nc.sync.dma_start(thrp_sb[:],
                                  thrp.rearrange("(a p) b -> p (a b)", p=128))
                nc.sync.dma_start(sigbp_sb[:],
                                  sigbp.rearrange("(a p) b -> p (a b)", p=128))
                nc.gpsimd.dma_start(idx_sb[0][:], idxrow[0, :, :])
                nc.gpsimd.dma_start(idx_sb[1][:], idxrow[1, :, :])
                nc.gpsimd.dma_start(mask_sb[:], maskr[:])

            # ---- per-pair x tiles: eighth 0 and 1 split (fast A start), then
            # quarters. Returns per-kind list of (tile, col_base) eighth views.
            def load_x(p):
                kinds = {"xt0": (xt[p, 0:128, :], F32R),
                         "xt1": (xt[p, 128:256, :], F32R),
                         "xb0": (xtb[p, 0:128, :], F16),
                         "xb1": (xtb[p, 128:256, :], F16)}
                views = {nm: [None] * 8 for nm in kinds}
                for part, (c0, c1) in enumerate(
                        [(0, 512), (512, 1024), (1024, 2048), (2048, 3072),
                         (3072, 4096)]):
                    for nm, (src, dt) in kinds.items():
                        t = xtp.tile([128, c1 - c0], dt, tag=f"{nm}p{part}",
                                     name=f"{nm}p{part}")
                        s = src[:, c0:c1]
                        if dt == F32R:
                            nc.sync.dma_start(t[:], s.bitcast(F32R))
                        else:
                            nc.gpsimd.dma_start(t[:], s)
                        for e in range(c0 // 512, c1 // 512):
                            views[nm][e] = t[:, e * 512 - c0:(e + 1) * 512 - c0]
                return views

            xq = [None, None]
            kv_sb = [None, None]
            kvw_sb = [None, None]
            kvg_sb = [None, None]
            qt_sb = [None, None]
            outT = [None, None]
            out8 = [None, None]

            def phase_AB(p):
                kv_sb[p] = big.tile([128, 32 * 512], F8, tag="kv", name="kv")
                kvw_sb[p] = big.tile([128, NW * 256], BF16, tag="kvw", name="kvw")
                nc.vector.memset(kvw_sb[p][:], 0.0)
                xqp = xq[p]
                for mp in range(16):
                    ps = psW.tile([128, 1024], F32, tag="psW", name="psW")
                    for h in range(2):
                        m = 2 * mp + h
                        e, mo = m // 4, (m % 4) * 128
                        msl = slice(mo, mo + 128)
                        psl = slice(h * 512, (h + 1) * 512)
                        nc.tensor.matmul(ps[:, psl], xqp["xt0"][e][:, msl],
                                         wkv_sb[0][:], start=True, stop=False,
                                         skip_group_check=True)
                        nc.tensor.matmul(ps[:, psl], xqp["xt0"][e][:, msl],
                                         wvkv_sb[0][:], start=False, stop=False,
                                         skip_group_check=True)
                        nc.tensor.matmul(ps[:, psl], xqp["xt1"][e][:, msl],
                                         wkv_sb[1][:], start=False, stop=False,
                                         skip_group_check=True)
                        nc.tensor.matmul(ps[:, psl], xqp["xt1"][e][:, msl],
                                         wvkv_sb[1][:], start=False, stop=False,
                                         skip_group_check=True)
                        nc.tensor.matmul(ps[:, psl], xqp["xb0"][e][:, msl],
                                         wkv16_sb[0][:], start=False, stop=False,
                                         skip_group_check=True)
                        nc.tensor.matmul(ps[:, psl], xqp["xb1"][e][:, msl],
                                         wkv16_sb[1][:], start=False, stop=True,
                                         skip_group_check=True)
                    nc.vector.tensor_tensor(
                        kv_sb[p][:, mp * 1024:(mp + 1) * 1024],
                        ps[:], thrkv_sb[:], op=GE)
                    # B round after every 4 mp (8 m-tiles): windows 2r, 2r+1
                    if mp % 4 == 3:
                        r = mp // 4
                        kvwf = psB.tile([128, 512], F32, tag="kvwf", name="kvwf")
                        for wl in range(2):
                            j = 2 * r + wl
                            base = j * 2048
                            kvv = kv_sb[p][:, base:base + 1024].rearrange(
                                "p (two x) -> p two x", two=2)
                            kvv2 = kv_sb[p][:, base + 1024:base + 2048].rearrange(
                                "p (two x) -> p two x", two=2)
                            for hp in range(2):
                                ksl = slice(hp * 128, hp * 128 + 128)
                                vsl = slice(256 + hp * 128, 256 + hp * 128 + 128)
                                blk = (2 * wl + hp) * 128
                                nc.tensor.matmul(
                                    kvwf[:, blk:blk + 128],
                                    kvv[:, :, ksl], kvv[:, :, vsl],
                                    start=(wl == 0 and hp == 0), stop=False,
                                    perf_mode=DRMODE, skip_group_check=True)
                                nc.tensor.matmul(
                                    kvwf[:, blk:blk + 128],
                                    kvv2[:, :, ksl], kvv2[:, :, vsl],
                                    start=False, stop=(wl == 1 and hp == 1),
                                    perf_mode=DRMODE, skip_group_check=True)
                        # extract diag [64,64] blocks -> kvw_sb (block-diag, zeros off-diag)
                        for s in range(2):
                            srows = slice(s * 64, (s + 1) * 64)
                            srcap = kvwf[srows, :].rearrange(
                                "q (b e) -> q b e", e=128)[:, :, s * 64:s * 64 + 64]
                            dstap = kvw_sb[p][srows, r * 512:(r + 1) * 512].rearrange(
                                "q (b e) -> q b e", e=128)[:, :, s * 64:s * 64 + 64]
                            if s == 0:
                                nc.vector.tensor_copy(dstap, srcap)
                            else:
                                nc.scalar.copy(dstap, srcap)
                        # write the 2 windows' kvw blocks to DRAM for the gather
                        rows = slice(2 * r * 128, (2 * r + 2) * 128)
                        nc.sync.dma_start(
                            kvw_dram[p][rows, :].rearrange("(j p) e -> p j e", p=128),
                            kvw_sb[p][:, r * 512:(r + 1) * 512])

            def phase_qT(p, extra_vec_ops=()):
                extra = list(extra_vec_ops)
                qt_sb[p] = big.tile([128, 2 * L], BF16, tag="qt", name="qt")
                xqp = xq[p]
                for g in range(8):
                    for dq in range(2):
                        for _ in range(3):
                            if extra:
                                extra.pop(0)()
                        ps = psN.tile([128, 512], F32, tag="psN", name="psQ")
                        dsl = slice(dq * 128, (dq + 1) * 128)
                        nc.tensor.matmul(ps[:], w_sb[0][:, dsl], xqp["xt0"][g],
                                         start=True, stop=False)
                        nc.tensor.matmul(ps[:], wq16_sb[0][:, dsl], xqp["xb0"][g],
                                         start=False, stop=False)
                        nc.tensor.matmul(ps[:], w_sb[1][:, dsl], xqp["xt1"][g],
                                         start=False, stop=False)
                        nc.tensor.matmul(ps[:], wq16_sb[1][:, dsl], xqp["xb1"][g],
                                         start=False, stop=False)
                        nc.tensor.matmul(ps[:], wv_sb[0][:, dsl], xqp["xt0"][g],
                                         start=False, stop=False)
                        nc.tensor.matmul(ps[:], wv_sb[1][:, dsl], xqp["xt1"][g],
                                         start=False, stop=True)
                        dst = qt_sb[p][:, dq * L + g * 512: dq * L + (g + 1) * 512]
                        if dq == 0:
                            nc.scalar.activation(dst, ps[:], SIG,
                                                 bias=sigbq_sb[:, 0:1], scale=BIGS)
                        else:
                            nc.vector.tensor_scalar(dst, ps[:], thrq_sb[:, 1:2],
                                                    None, GE)

            kvgB = [None]

            def phase_agg(p, nlist):
                # 4 gathers per target window: slot 0 initializes, 1-3 accumulate
                if kvg_sb[p] is None:
                    kvg_sb[p] = dbl.tile([128, NW * 256], BF16, tag="kvg",
                                         name="kvg")
                for n in nlist:
                    dst = kvg_sb[p][:, n * 256:(n + 1) * 256]
                    for i in range(TOPK):
                        m = n * TOPK + i
                        nc.gpsimd.indirect_dma_start(
                            out=dst, out_offset=None,
                            in_=kvw_dram[p][:],
                            in_offset=bass.IndirectOffsetOnAxis(
                                ap=idx_sb[p][:, m:m + 1], axis=0),
                            compute_op=(ADD if i > 0 else mybir.AluOpType.bypass),
                        )

            def agg_stt_ops(p, nlist):
                # mask-weighted accumulation on the vector engine for pair p's
                # windows in nlist, written to the separate kvgB tile. Returns
                # a list of closures to be emitted interleaved with other work.
                kvgB[0] = big.tile([128, 4 * 256], BF16, tag="kvgb", name="kvgb")
                ops = []
                for ni, n in enumerate(nlist):
                    dst = kvgB[0][:, ni * 256:(ni + 1) * 256]
                    ops.append(lambda dst=dst, n=n: nc.vector.tensor_scalar(
                        dst, kvw_sb[p][:, 0:256], mask_sb[:, n * 8:n * 8 + 1],
                        None, MULT))
                    for w in range(1, NW):
                        ops.append(lambda dst=dst, n=n, w=w:
                                   nc.vector.scalar_tensor_tensor(
                                       dst, kvw_sb[p][:, w * 256:(w + 1) * 256],
                                       mask_sb[:, n * 8 + w:n * 8 + w + 1],
                                       dst, op0=MULT, op1=ADD))
                return ops

            def phase_C(p):
                outT[p] = big.tile([128, 2 * L], F32R, tag="ot", name="ot")
                out8[p] = big.tile([128, 2 * L], F8, tag="o8", name="o8")
                otv = outT[p][:].rearrange("p (two l) -> p two l", two=2)
                o8v = out8[p][:].rearrange("p (two l) -> p two l", two=2)
                for n in range(NW):
                    ps = psW.tile([128, 1024], F32, tag="psW", name="psC")
                    if p == 1 and n >= 4:
                        kvgsrc = kvgB[0][:, (n - 4) * 256:(n - 3) * 256]
                    else:
                        kvgsrc = kvg_sb[p][:, n * 256:(n + 1) * 256]
                    for dq in range(2):
                        nc.tensor.matmul(
                            ps[:, dq * 512:(dq + 1) * 512],
                            kvgsrc[:, dq * 128:dq * 128 + 128],
                            qt_sb[p][:, dq * L + n * 512: dq * L + (n + 1) * 512],
                            start=True, stop=True, skip_group_check=True)
                    nsl = slice(n * 512, (n + 1) * 512)
                    nc.vector.tensor_copy(otv[:, :, nsl], ps[:])
                    nc.scalar.activation(o8v[:, :, nsl], ps[:], COPYF, scale=0.03125)

            def phase_D(p):
                w8v = wpl8_sb[:].rearrange("p (two oc) -> p two oc", two=2)
                o8v = out8[p][:].rearrange("p (two x) -> p two x", two=2)
                for g in range(8):
                    gsl = slice(g * 512, (g + 1) * 512)
                    for ct in range(2):
                        ps = psN.tile([128, 512], F32, tag="psN", name="psD")
                        csl = slice(ct * 128, (ct + 1) * 128)
                        nc.tensor.matmul(ps[:], wp_sb[0][:, csl],
                                         outT[p][:, gsl],
                                         start=True, stop=False,
                                         skip_group_check=True)
                        nc.tensor.matmul(ps[:], wp_sb[1][:, csl],
                                         outT[p][:, L + g * 512: L + (g + 1) * 512],
                                         start=False, stop=False,
                                         skip_group_check=True)
                        for sub in range(4):
                            nc.tensor.matmul(
                                ps[:, sub * 128:(sub + 1) * 128],
                                w8v[:, :, csl],
                                o8v[:, :, g * 512 + sub * 128: g * 512 + (sub + 1) * 128],
                                start=False, stop=(sub == 3),
                                perf_mode=DRMODE, skip_group_check=True)
                        fin = finp.tile([128, 512], F8, tag="fin", name="fin")
                        if ct == 0:
                            nc.scalar.activation(fin[:], ps[:], SIG,
                                                 bias=sigbp_sb[:, 0:1], scale=BIGS)
                        else:
                            nc.vector.tensor_scalar(fin[:], ps[:], thrp_sb[:, 1:2],
                                                    None, GE)
                        nc.scalar.dma_start(
                            out[p, ct * 128:(ct + 1) * 128, g * 512:(g + 1) * 512],
                            fin[:])

            # ---- zippered emission: both AB phases first so the gather DMAs
            # (the long pole on the gpsimd queue) start as early as possible.
            # Pair 1's windows 4-7 aggregate on the vector engine instead,
            # interleaved with qT0's evacuations. ----
            xq[0] = load_x(0)
            load_consts_rest()
            xq[1] = load_x(1)
            phase_AB(0)
            phase_agg(0, range(NW))
            phase_AB(1)
            phase_agg(1, range(4))
            phase_qT(0, agg_stt_ops(1, range(4, NW)))
            phase_C(0)
            phase_D(0)
            phase_qT(1)
            phase_C(1)
            phase_D(1)

    nc.compile()
    return nc


_NC = None


def _f32r_round(a):
    """Round fp32 to the f32r grid (12-bit significand, round-to-nearest)."""
    u = np.ascontiguousarray(a, dtype=np.float32).view(np.uint32)
    u = (u + np.uint32(1 << 11)) & np.uint32(0xFFFFF000)
    return u.view(np.float32)


def kernel(x, W_qkv, b_qkv, W_proj, b_proj):
    global _NC, _EXEC_TIME_NS
    x = np.asarray(x, dtype=np.float32)
    W_qkv = np.asarray(W_qkv, dtype=np.float32)
    b_qkv = np.asarray(b_qkv, dtype=np.float32)
    W_proj = np.asarray(W_proj, dtype=np.float32)
    b_proj = np.asarray(b_proj, dtype=np.float32)

    # ---- host routing: region sums -> attn -> top-k window indices ----
    region = x.sum(axis=0).reshape(B, NW, WIN, C).sum(axis=2)        # [B,NW,C]
    attn_r = np.einsum('bnc,bmc->bnm', region, region)
    idx = np.argsort(-attn_r, axis=-1, kind='stable')[:, :, :TOPK]   # [B,NW,TOPK]
    masks = np.zeros((B, NW, NW), np.float32)
    for b in range(B):
        for n in range(NW):
            masks[b, n, idx[b, n]] = 1.0

    # ---- common (replicated) inputs ----
    wq_u = _f32r_round(W_qkv)
    wp_u = _f32r_round(W_proj)
    wpl = (W_proj - wp_u) * 16384.0
    wpl8_np = np.empty((128, 512), dtype=ml_dtypes.float8_e4m3)
    wpl8_np[:, 0:256] = wpl[0:128, :].astype(ml_dtypes.float8_e4m3)
    wpl8_np[:, 256:512] = wpl[128:256, :].astype(ml_dtypes.float8_e4m3)
    thrkv1 = 2.0 - b_qkv[None, 256:768]
    common = {
        "wqkv": wq_u,
        "wqv": np.ascontiguousarray(W_qkv - wq_u),
        "wkv16": np.ascontiguousarray(wq_u[:, 256:768]).astype(np.float16),
        "wq16": np.ascontiguousarray(wq_u[:, 0:256]).astype(np.float16),
        "thrkv2": np.ascontiguousarray(
            np.broadcast_to(np.tile(thrkv1, (1, 2)), (128, 1024))),
        "thrq": np.ascontiguousarray(2.0 - b_qkv[0:256, None]),
        "sigbq": np.ascontiguousarray(
            -BIGS * (2.0 - b_qkv[0:256, None])).astype(np.float32),
        "wps": np.ascontiguousarray(wp_u * PS),
        "wpl8": wpl8_np,
        "thrp": np.ascontiguousarray(PS * (2.0 - b_proj[:, None])),
        "sigbp": np.ascontiguousarray(
            -BIGS * PS * (2.0 - b_proj[:, None])).astype(np.float32),
    }

    in_maps = []
    pairs = [(t, b) for t in range(T) for b in range(B)]
    for core in range(NCORES):
        mine = pairs[core * NPAIR:(core + 1) * NPAIR]
        xt_full = np.stack([np.ascontiguousarray(x[t, b].T) for (t, b) in mine])
        xt = _f32r_round(xt_full)
        rows = []
        for k, (t, b) in enumerate(mine):
            r = np.empty((128, NW * TOPK), dtype=np.int32)
            for n in range(NW):
                for i in range(TOPK):
                    r[:, n * TOPK + i] = idx[b, n, i] * 128 + np.arange(128)
            rows.append(r)
        m = dict(common)
        m["xt"] = xt
        m["xtb"] = (xt_full - xt).astype(np.float16)
        m["idxrow"] = np.stack(rows)
        b1 = mine[1][1]
        m["maskr"] = np.ascontiguousarray(np.broadcast_to(
            masks[b1].reshape(1, NW * NW), (128, NW * NW)))
        in_maps.append(m)

    if _NC is None:
        _NC = _build_nc()

    traceable = _ensure_ntff_hook()
    try:
        res = bass_utils.run_bass_kernel_spmd(_NC, in_maps,
                                              core_ids=list(range(NCORES)),
                                              trace=traceable)
    except Exception:
        if not traceable:
            raise
        res = bass_utils.run_bass_kernel_spmd(_NC, in_maps,
                                              core_ids=list(range(NCORES)),
                                              trace=False)
    _EXEC_TIME_NS = res.exec_time_ns

    full = np.empty((T, B, L, C), dtype=np.float32)
    for core in range(NCORES):
        mine = pairs[core * NPAIR:(core + 1) * NPAIR]
        o = np.asarray(res.results[core]["out"]).astype(np.float32)  # [NPAIR,C,L]
        for k, (t, b) in enumerate(mine):
            full[t, b] = o[k].T
    return full


# revision 47
# speedup vs baseline: 1.0212x; 1.0212x over previous
"""BiLevelRoutingAttention (spiking, linear attention with window routing) on 8 TRN2 cores.

Sharding: 16 (t,b) pairs -> 2 per core, data-parallel. Host precomputes routing
(region sums -> top-k window indices -> gather row indices); the device does
qkv projection (3-term f32r residual-split for fp32-grade accuracy, thresholds
fused into paired-PSUM-bank evacuations), per-window kv outer products (fp8
DoubleRow on the binary spikes), top-k aggregation via indirect-DMA row gathers
with CCE-add accumulation (runs on the otherwise-idle gpsimd DMA queue),
linear attention (bf16), and the output projection (2-term: f32r main + fp8
DoubleRow residual at a 2^9 PSUM scale), producing binary spikes in fp8 that
the host converts back to f32 and transposes. The two pairs per core are
software-pipelined so the tensor engine never drains.
"""
import sys
sys.path.insert(0, '/opt/trn_rl_repo')

import numpy as np
import ml_dtypes

import concourse.bass as bass
import concourse.bacc as bacc
import concourse.mybir as mybir
from concourse.tile import TileContext
from concourse import bass_utils

F32 = mybir.dt.float32
F32R = mybir.dt.float32r
BF16 = mybir.dt.bfloat16
F8 = mybir.dt.float8e4
F16 = mybir.dt.float16
I32 = mybir.dt.int32
GE = mybir.AluOpType.is_ge
MULT = mybir.AluOpType.mult
ADD = mybir.AluOpType.add
SIG = mybir.ActivationFunctionType.Sigmoid
COPYF = mybir.ActivationFunctionType.Copy
DRMODE = mybir.MatmulPerfMode.DoubleRow

T, B, L, C = 4, 4, 4096, 256
NW, TOPK, H, D = 8, 4, 4, 64
WIN = L // NW           # 512
NCORES = 8
NPAIR = 2               # (t,b) pairs per core
BIGS = 1.0e18           # sigmoid saturation scale
PS = 512.0              # proj main-term PSUM scale (= 2^-5 * 2^14)

_EXEC_TIME_NS = None    # stashed for test harness


def _ensure_ntff_hook():
    """The agent image's antenv lacks axon_hooks; register the same hook
    trn_boot would have installed so trace=True can collect NTFF profiles."""
    import types
    try:
        import antenv.axon_hooks  # noqa: F401
        return True
    except ImportError:
        pass
    try:
        import antenv
        from trn_agent_boot.trn_boot import _ntff_profile_via_ctypes
        state = {"hook": _ntff_profile_via_ctypes('/opt/axon/libaxon_pjrt.so')}
        mod = types.ModuleType("antenv.axon_hooks")
        mod.get_axon_ntff_profile_hook = lambda: state["hook"]
        mod.set_axon_ntff_profile_hook = lambda h: state.__setitem__("hook", h)
        sys.modules["antenv.axon_hooks"] = mod
        antenv.axon_hooks = mod
        return True
    except Exception:
        return False


def _build_nc():
    nc = bacc.Bacc("TRN2", target_bir_lowering=False, debug=False,
                   num_devices=8)

    xt = nc.dram_tensor("xt", [NPAIR, C, L], F32, kind="ExternalInput")
    xtb = nc.dram_tensor("xtb", [NPAIR, C, L], F16, kind="ExternalInput")
    wqkv = nc.dram_tensor("wqkv", [C, 768], F32, kind="ExternalInput")
    wqv = nc.dram_tensor("wqv", [C, 768], F32, kind="ExternalInput")
    wkv16 = nc.dram_tensor("wkv16", [C, 512], F16, kind="ExternalInput")
    wq16 = nc.dram_tensor("wq16", [C, 256], F16, kind="ExternalInput")
    thrkv2 = nc.dram_tensor("thrkv2", [128, 1024], F32, kind="ExternalInput")
    thrq = nc.dram_tensor("thrq", [C, 1], F32, kind="ExternalInput")
    sigbq = nc.dram_tensor("sigbq", [C, 1], F32, kind="ExternalInput")
    wps = nc.dram_tensor("wps", [C, C], F32, kind="ExternalInput")
    wpl8 = nc.dram_tensor("wpl8", [128, 512], F8, kind="ExternalInput")
    thrp = nc.dram_tensor("thrp", [C, 1], F32, kind="ExternalInput")
    sigbp = nc.dram_tensor("sigbp", [C, 1], F32, kind="ExternalInput")
    idxrow = nc.dram_tensor("idxrow", [NPAIR, 128, NW * TOPK], I32,
                            kind="ExternalInput")
    maskr = nc.dram_tensor("maskr", [128, NW * NW], F32, kind="ExternalInput")
    out = nc.dram_tensor("out", [NPAIR, C, L], F8, kind="ExternalOutput")
    kvw_dram = [nc.dram_tensor("kvw_scratch0", [NW * 128, 256], BF16,
                               kind="Internal"),
                nc.dram_tensor("kvw_scratch1", [NW * 128, 256], BF16,
                               kind="Internal")]

    with TileContext(nc) as tc:
        with (
            tc.tile_pool(name="const", bufs=1) as cpool,
            tc.tile_pool(name="xtp", bufs=2) as xtp,
            tc.tile_pool(name="big", bufs=1) as big,
            tc.tile_pool(name="dbl", bufs=2) as dbl,
            tc.tile_pool(name="fin", bufs=4) as finp,
            tc.tile_pool(name="psW", bufs=2, space="PSUM") as psW,
            tc.tile_pool(name="psN", bufs=2, space="PSUM") as psN,
            tc.tile_pool(name="psB", bufs=2, space="PSUM") as psB,
        ):
            # ---- constants / weights ----
            # k/v columns (needed by phase A immediately) in their own tiles;
            # q columns (needed only by qT, much later) loaded separately.
            wkv_sb = [cpool.tile([128, 512], F32R, tag="wkv0", name="wkv0"),
                      cpool.tile([128, 512], F32R, tag="wkv1", name="wkv1")]
            nc.gpsimd.dma_start(wkv_sb[0][:], wqkv[0:128, 256:768])
            nc.gpsimd.dma_start(wkv_sb[1][:], wqkv[128:256, 256:768])
            wvkv_sb = [cpool.tile([128, 512], F32R, tag="wvkv0", name="wvkv0"),
                       cpool.tile([128, 512], F32R, tag="wvkv1", name="wvkv1")]
            nc.gpsimd.dma_start(wvkv_sb[0][:], wqv[0:128, 256:768])
            nc.gpsimd.dma_start(wvkv_sb[1][:], wqv[128:256, 256:768])
            wkv16_sb = [cpool.tile([128, 512], F16, tag="wkv16_0", name="wkv16_0"),
                        cpool.tile([128, 512], F16, tag="wkv16_1", name="wkv16_1")]
            nc.gpsimd.dma_start(wkv16_sb[0][:], wkv16[0:128, :])
            nc.gpsimd.dma_start(wkv16_sb[1][:], wkv16[128:256, :])
            thrkv_sb = cpool.tile([128, 1024], F32, tag="thrkv", name="thrkv")
            nc.gpsimd.dma_start(thrkv_sb[:], thrkv2[:])
            w_sb = [cpool.tile([128, 256], F32R, tag="wq0", name="wq0"),
                    cpool.tile([128, 256], F32R, tag="wq1", name="wq1")]
            wv_sb = [cpool.tile([128, 256], F32R, tag="wv0", name="wv0"),
                     cpool.tile([128, 256], F32R, tag="wv1", name="wv1")]
            wq16_sb = [cpool.tile([128, 256], F16, tag="wq16_0", name="wq16_0"),
                       cpool.tile([128, 256], F16, tag="wq16_1", name="wq16_1")]
            wp_sb = [cpool.tile([128, 256], F32R, tag="wp0", name="wp0"),
                     cpool.tile([128, 256], F32R, tag="wp1", name="wp1")]
            wpl8_sb = cpool.tile([128, 512], F8, tag="wpl8", name="wpl8")
            thrq_sb = cpool.tile([128, 2], F32, tag="thrq", name="thrq")
            sigbq_sb = cpool.tile([128, 2], F32, tag="sigbq", name="sigbq")
            thrp_sb = cpool.tile([128, 2], F32, tag="thrp", name="thrp")
            sigbp_sb = cpool.tile([128, 2], F32, tag="sigbp", name="sigbp")
            idx_sb = [cpool.tile([128, NW * TOPK], I32, tag="idx0", name="idx0"),
                      cpool.tile([128, NW * TOPK], I32, tag="idx1", name="idx1")]
            mask_sb = cpool.tile([128, NW * NW], F32, tag="maskr", name="maskr")

            def load_consts_rest():
                nc.gpsimd.dma_start(w_sb[0][:], wqkv[0:128, 0:256])
                nc.gpsimd.dma_start(w_sb[1][:], wqkv[128:256, 0:256])
                nc.gpsimd.dma_start(wv_sb[0][:], wqv[0:128, 0:256])
                nc.gpsimd.dma_start(wv_sb[1][:], wqv[128:256, 0:256])
                nc.gpsimd.dma_start(wq16_sb[0][:], wq16[0:128, :])
                nc.gpsimd.dma_start(wq16_sb[1][:], wq16[128:256, :])
                nc.gpsimd.dma_start(wp_sb[0][:], wps[0:128, :])
                nc.gpsimd.dma_start(wp_sb[1][:], wps[128:256, :])
                nc.gpsimd.dma_start(wpl8_sb[:], wpl8[:])
                nc.sync.dma_start(thrq_sb[:],
                                  thrq.rearrange("(a p) b -> p (a b)", p=128))
                nc.sync.dma_start(sigbq_sb[:],
                                  sigbq.rearrange("(a p) b -> p (a b)", p=128))
                nc.sync.dma_start(thrp_sb[:],
                                  thrp.rearrange("(a p) b -> p (a b)", p=128))
                nc.sync.dma_start(sigbp_sb[:],
                                  sigbp.rearrange("(a p) b -> p (a b)", p=128))
                nc.gpsimd.dma_start(idx_sb[0][:], idxrow[0, :, :])
                nc.gpsimd.dma_start(idx_sb[1][:], idxrow[1, :, :])
                nc.gpsimd.dma_start(mask_sb[:], maskr[:])

            # ---- per-pair x tiles: eighth 0 and 1 split (fast A start), then
            # quarters. Returns per-kind list of (tile, col_base) eighth views.
            def load_x(p):
                kinds = {"xt0": (xt[p, 0:128, :], F32R),
                         "xt1": (xt[p, 128:256, :], F32R),
                         "xb0": (xtb[p, 0:128, :], F16),
                         "xb1": (xtb[p, 128:256, :], F16)}
                views = {nm: [None] * 8 for nm in kinds}
                for part, (c0, c1) in enumerate(
                        [(0, 512), (512, 1024), (1024, 2048), (2048, 3072),
                         (3072, 4096)]):
                    for nm, (src, dt) in kinds.items():
                        t = xtp.tile([128, c1 - c0], dt, tag=f"{nm}p{part}",
                                     name=f"{nm}p{part}")
                        s = src[:, c0:c1]
                        nc.sync.dma_start(t[:], s.bitcast(F32R) if dt == F32R else s)
                        for e in range(c0 // 512, c1 // 512):
                            views[nm][e] = t[:, e * 512 - c0:(e + 1) * 512 - c0]
                return views

            xq = [None, None]
            kv_sb = [None, None]
            kvw_sb = [None, None]
            kvg_sb = [None, None]
            qt_sb = [None, None]
            outT = [None, None]
            out8 = [None, None]

            def phase_AB(p):
                kv_sb[p] = big.tile([128, 32 * 512], F8, tag="kv", name="kv")
                kvw_sb[p] = big.tile([128, NW * 256], BF16, tag="kvw", name="kvw")
                nc.vector.memset(kvw_sb[p][:], 0.0)
                xqp = xq[p]
                for mp in range(16):
                    ps = psW.tile([128, 1024], F32, tag="psW", name="psW")
                    for h in range(2):
                        m = 2 * mp + h
                        e, mo = m // 4, (m % 4) * 128
                        msl = slice(mo, mo + 128)
                        psl = slice(h * 512, (h + 1) * 512)
                        nc.tensor.matmul(ps[:, psl], xqp["xt0"][e][:, msl],
                                         wkv_sb[0][:], start=True, stop=False,
                                         skip_group_check=True)
                        nc.tensor.matmul(ps[:, psl], xqp["xt0"][e][:, msl],
                                         wvkv_sb[0][:], start=False, stop=False,
                                         skip_group_check=True)
                        nc.tensor.matmul(ps[:, psl], xqp["xt1"][e][:, msl],
                                         wkv_sb[1][:], start=False, stop=False,
                                         skip_group_check=True)
                        nc.tensor.matmul(ps[:, psl], xqp["xt1"][e][:, msl],
                                         wvkv_sb[1][:], start=False, stop=False,
                                         skip_group_check=True)
                        nc.tensor.matmul(ps[:, psl], xqp["xb0"][e][:, msl],
                                         wkv16_sb[0][:], start=False, stop=False,
                                         skip_group_check=True)
                        nc.tensor.matmul(ps[:, psl], xqp["xb1"][e][:, msl],
                                         wkv16_sb[1][:], start=False, stop=True,
                                         skip_group_check=True)
                    nc.vector.tensor_tensor(
                        kv_sb[p][:, mp * 1024:(mp + 1) * 1024],
                        ps[:], thrkv_sb[:], op=GE)
                    # B round after every 4 mp (8 m-tiles): windows 2r, 2r+1
                    if mp % 4 == 3:
                        r = mp // 4
                        kvwf = psB.tile([128, 512], F32, tag="kvwf", name="kvwf")
                        for wl in range(2):
                            j = 2 * r + wl
                            base = j * 2048
                            kvv = kv_sb[p][:, base:base + 1024].rearrange(
                                "p (two x) -> p two x", two=2)
                            kvv2 = kv_sb[p][:, base + 1024:base + 2048].rearrange(
                                "p (two x) -> p two x", two=2)
                            for hp in range(2):
                                ksl = slice(hp * 128, hp * 128 + 128)
                                vsl = slice(256 + hp * 128, 256 + hp * 128 + 128)
                                blk = (2 * wl + hp) * 128
                                nc.tensor.matmul(
                                    kvwf[:, blk:blk + 128],
                                    kvv[:, :, ksl], kvv[:, :, vsl],
                                    start=(wl == 0 and hp == 0), stop=False,
                                    perf_mode=DRMODE, skip_group_check=True)
                                nc.tensor.matmul(
                                    kvwf[:, blk:blk + 128],
                                    kvv2[:, :, ksl], kvv2[:, :, vsl],
                                    start=False, stop=(wl == 1 and hp == 1),
                                    perf_mode=DRMODE, skip_group_check=True)
                        # extract diag [64,64] blocks -> kvw_sb (block-diag, zeros off-diag)
                        for s in range(2):
                            srows = slice(s * 64, (s + 1) * 64)
                            srcap = kvwf[srows, :].rearrange(
                                "q (b e) -> q b e", e=128)[:, :, s * 64:s * 64 + 64]
                            dstap = kvw_sb[p][srows, r * 512:(r + 1) * 512].rearrange(
                                "q (b e) -> q b e", e=128)[:, :, s * 64:s * 64 + 64]
                            if s == 0:
                                nc.vector.tensor_copy(dstap, srcap)
                            else:
                                nc.scalar.copy(dstap, srcap)
                        # write the 2 windows' kvw blocks to DRAM for the gather
                        rows = slice(2 * r * 128, (2 * r + 2) * 128)
                        nc.sync.dma_start(
                            kvw_dram[p][rows, :].rearrange("(j p) e -> p j e", p=128),
                            kvw_sb[p][:, r * 512:(r + 1) * 512])

            def phase_qT(p, extra_vec_ops=()):
                extra = list(extra_vec_ops)
                qt_sb[p] = big.tile([128, 2 * L], BF16, tag="qt", name="qt")
                xqp = xq[p]
                for g in range(8):
                    for dq in range(2):
                        for _ in range(3):
                            if extra:
                                extra.pop(0)()
                        ps = psN.tile([128, 512], F32, tag="psN", name="psQ")
                        dsl = slice(dq * 128, (dq + 1) * 128)
                        nc.tensor.matmul(ps[:], w_sb[0][:, dsl], xqp["xt0"][g],
                                         start=True, stop=False)
                        nc.tensor.matmul(ps[:], wq16_sb[0][:, dsl], xqp["xb0"][g],
                                         start=False, stop=False)
                        nc.tensor.matmul(ps[:], w_sb[1][:, dsl], xqp["xt1"][g],
                                         start=False, stop=False)
                        nc.tensor.matmul(ps[:], wq16_sb[1][:, dsl], xqp["xb1"][g],
                                         start=False, stop=False)
                        nc.tensor.matmul(ps[:], wv_sb[0][:, dsl], xqp["xt0"][g],
                                         start=False, stop=False)
                        nc.tensor.matmul(ps[:], wv_sb[1][:, dsl], xqp["xt1"][g],
                                         start=False, stop=True)
                        dst = qt_sb[p][:, dq * L + g * 512: dq * L + (g + 1) * 512]
                        if dq == 0:
                            nc.scalar.activation(dst, ps[:], SIG,
                                                 bias=sigbq_sb[:, 0:1], scale=BIGS)
                        else:
                            nc.vector.tensor_scalar(dst, ps[:], thrq_sb[:, 1:2],
                                                    None, GE)

            kvgB = [None]

            def phase_agg(p, nlist):
                # 4 gathers per target window: slot 0 initializes, 1-3 accumulate
                if kvg_sb[p] is None:
                    kvg_sb[p] = dbl.tile([128, NW * 256], BF16, tag="kvg",
                                         name="kvg")
                for n in nlist:
                    dst = kvg_sb[p][:, n * 256:(n + 1) * 256]
                    for i in range(TOPK):
                        m = n * TOPK + i
                        nc.gpsimd.indirect_dma_start(
                            out=dst, out_offset=None,
                            in_=kvw_dram[p][:],
                            in_offset=bass.IndirectOffsetOnAxis(
                                ap=idx_sb[p][:, m:m + 1], axis=0),
                            compute_op=(ADD if i > 0 else mybir.AluOpType.bypass),
                        )

            def agg_stt_ops(p, nlist):
                # mask-weighted accumulation on the vector engine for pair p's
                # windows in nlist, written to the separate kvgB tile. Returns
                # a list of closures to be emitted interleaved with other work.
                kvgB[0] = big.tile([128, 4 * 256], BF16, tag="kvgb", name="kvgb")
                ops = []
                for ni, n in enumerate(nlist):
                    dst = kvgB[0][:, ni * 256:(ni + 1) * 256]
                    ops.append(lambda dst=dst, n=n: nc.vector.tensor_scalar(
                        dst, kvw_sb[p][:, 0:256], mask_sb[:, n * 8:n * 8 + 1],
                        None, MULT))
                    for w in range(1, NW):
                        ops.append(lambda dst=dst, n=n, w=w:
                                   nc.vector.scalar_tensor_tensor(
                                       dst, kvw_sb[p][:, w * 256:(w + 1) * 256],
                                       mask_sb[:, n * 8 + w:n * 8 + w + 1],
                                       dst, op0=MULT, op1=ADD))
                return ops

            def phase_C(p):
                outT[p] = big.tile([128, 2 * L], F32R, tag="ot", name="ot")
                out8[p] = big.tile([128, 2 * L], F8, tag="o8", name="o8")
                otv = outT[p][:].rearrange("p (two l) -> p two l", two=2)
                o8v = out8[p][:].rearrange("p (two l) -> p two l", two=2)
                for n in range(NW):
                    ps = psW.tile([128, 1024], F32, tag="psW", name="psC")
                    if p == 1 and n >= 4:
                        kvgsrc = kvgB[0][:, (n - 4) * 256:(n - 3) * 256]
                    else:
                        kvgsrc = kvg_sb[p][:, n * 256:(n + 1) * 256]
                    for dq in range(2):
                        nc.tensor.matmul(
                            ps[:, dq * 512:(dq + 1) * 512],
                            kvgsrc[:, dq * 128:dq * 128 + 128],
                            qt_sb[p][:, dq * L + n * 512: dq * L + (n + 1) * 512],
                            start=True, stop=True, skip_group_check=True)
                    nsl = slice(n * 512, (n + 1) * 512)
                    nc.vector.tensor_copy(otv[:, :, nsl], ps[:])
                    nc.scalar.activation(o8v[:, :, nsl], ps[:], COPYF, scale=0.03125)

            def phase_D(p):
                w8v = wpl8_sb[:].rearrange("p (two oc) -> p two oc", two=2)
                o8v = out8[p][:].rearrange("p (two x) -> p two x", two=2)
                for g in range(8):
                    gsl = slice(g * 512, (g + 1) * 512)
                    for ct in range(2):
                        ps = psN.tile([128, 512], F32, tag="psN", name="psD")
                        csl = slice(ct * 128, (ct + 1) * 128)
                        nc.tensor.matmul(ps[:], wp_sb[0][:, csl],
                                         outT[p][:, gsl],
                                         start=True, stop=False,
                                         skip_group_check=True)
                        nc.tensor.matmul(ps[:], wp_sb[1][:, csl],
                                         outT[p][:, L + g * 512: L + (g + 1) * 512],
                                         start=False, stop=False,
                                         skip_group_check=True)
                        for sub in range(4):
                            nc.tensor.matmul(
                                ps[:, sub * 128:(sub + 1) * 128],
                                w8v[:, :, csl],
                                o8v[:, :, g * 512 + sub * 128: g * 512 + (sub + 1) * 128],
                                start=False, stop=(sub == 3),
                                perf_mode=DRMODE, skip_group_check=True)
                        fin = finp.tile([128, 512], F8, tag="fin", name="fin")
                        if ct == 0:
                            nc.scalar.activation(fin[:], ps[:], SIG,
                                                 bias=sigbp_sb[:, 0:1], scale=BIGS)
                        else:
                            nc.vector.tensor_scalar(fin[:], ps[:], thrp_sb[:, 1:2],
                                                    None, GE)
                        nc.scalar.dma_start(
                            out[p, ct * 128:(ct + 1) * 128, g * 512:(g + 1) * 512],
                            fin[:])

            # ---- zippered emission: both AB phases first so the gather DMAs
            # (the long pole on the gpsimd queue) start as early as possible.
            # Pair 1's windows 4-7 aggregate on the vector engine instead,
            # interleaved with qT0's evacuations. ----
            xq[0] = load_x(0)
            load_consts_rest()
            xq[1] = load_x(1)
            phase_AB(0)
            phase_agg(0, range(NW))
            phase_AB(1)
            phase_agg(1, range(4))
            phase_qT(0, agg_stt_ops(1, range(4, NW)))
            phase_C(0)
            phase_D(0)
            phase_qT(1)
            phase_C(1)
            phase_D(1)

    nc.compile()
    return nc


_NC = None


def _f32r_round(a):
    """Round fp32 to the f32r grid (12-bit significand, round-to-nearest)."""
    u = np.ascontiguousarray(a, dtype=np.float32).view(np.uint32)
    u = (u + np.uint32(1 << 11)) & np.uint32(0xFFFFF000)
    return u.view(np.float32)


def kernel(x, W_qkv, b_qkv, W_proj, b_proj):
    global _NC, _EXEC_TIME_NS
    x = np.asarray(x, dtype=np.float32)
    W_qkv = np.asarray(W_qkv, dtype=np.float32)
    b_qkv = np.asarray(b_qkv, dtype=np.float32)
    W_proj = np.asarray(W_proj, dtype=np.float32)
    b_proj = np.asarray(b_proj, dtype=np.float32)

    # ---- host routing: region sums -> attn -> top-k window indices ----
    region = x.sum(axis=0).reshape(B, NW, WIN, C).sum(axis=2)        # [B,NW,C]
    attn_r = np.einsum('bnc,bmc->bnm', region, region)
    idx = np.argsort(-attn_r, axis=-1, kind='stable')[:, :, :TOPK]   # [B,NW,TOPK]
    masks = np.zeros((B, NW, NW), np.float32)
    for b in range(B):
        for n in range(NW):
            masks[b, n, idx[b, n]] = 1.0

    # ---- common (replicated) inputs ----
    wq_u = _f32r_round(W_qkv)
    wp_u = _f32r_round(W_proj)
    wpl = (W_proj - wp_u) * 16384.0
    wpl8_np = np.empty((128, 512), dtype=ml_dtypes.float8_e4m3)
    wpl8_np[:, 0:256] = wpl[0:128, :].astype(ml_dtypes.float8_e4m3)
    wpl8_np[:, 256:512] = wpl[128:256, :].astype(ml_dtypes.float8_e4m3)
    thrkv1 = 2.0 - b_qkv[None, 256:768]
    common = {
        "wqkv": wq_u,
        "wqv": np.ascontiguousarray(W_qkv - wq_u),
        "wkv16": np.ascontiguousarray(wq_u[:, 256:768]).astype(np.float16),
        "wq16": np.ascontiguousarray(wq_u[:, 0:256]).astype(np.float16),
        "thrkv2": np.ascontiguousarray(
            np.broadcast_to(np.tile(thrkv1, (1, 2)), (128, 1024))),
        "thrq": np.ascontiguousarray(2.0 - b_qkv[0:256, None]),
        "sigbq": np.ascontiguousarray(
            -BIGS * (2.0 - b_qkv[0:256, None])).astype(np.float32),
        "wps": np.ascontiguousarray(wp_u * PS),
        "wpl8": wpl8_np,
        "thrp": np.ascontiguousarray(PS * (2.0 - b_proj[:, None])),
        "sigbp": np.ascontiguousarray(
            -BIGS * PS * (2.0 - b_proj[:, None])).astype(np.float32),
    }

    in_maps = []
    pairs = [(t, b) for t in range(T) for b in range(B)]
    for core in range(NCORES):
        mine = pairs[core * NPAIR:(core + 1) * NPAIR]
        xt_full = np.stack([np.ascontiguousarray(x[t, b].T) for (t, b) in mine])
        xt = _f32r_round(xt_full)
        rows = []
        for k, (t, b) in enumerate(mine):
            r = np.empty((128, NW * TOPK), dtype=np.int32)
            for n in range(NW):
                for i in range(TOPK):
                    r[:, n * TOPK + i] = idx[b, n, i] * 128 + np.arange(128)
            rows.append(r)
        m = dict(common)
        m["xt"] = xt
        m["xtb"] = (xt_full - xt).astype(np.float16)
        m["idxrow"] = np.stack(rows)
        b1 = mine[1][1]
        m["maskr"] = np.ascontiguousarray(np.broadcast_to(
            masks[b1].reshape(1, NW * NW), (128, NW * NW)))
        in_maps.append(m)

    if _NC is None:
        _NC = _build_nc()

    traceable = _ensure_ntff_hook()
    try:
        res = bass_utils.run_bass_kernel_spmd(_NC, in_maps,
                                              core_ids=list(range(NCORES)),
                                              trace=traceable)
    except Exception:
        if not traceable:
            raise
        res = bass_utils.run_bass_kernel_spmd(_NC, in_maps,
                                              core_ids=list(range(NCORES)),
                                              trace=False)
    _EXEC_TIME_NS = res.exec_time_ns

    full = np.empty((T, B, L, C), dtype=np.float32)
    for core in range(NCORES):
        mine = pairs[core * NPAIR:(core + 1) * NPAIR]
        o = np.asarray(res.results[core]["out"]).astype(np.float32)  # [NPAIR,C,L]
        for k, (t, b) in enumerate(mine):
            full[t, b] = o[k].T
    return full
